# revision 18
# baseline (speedup 1.0000x reference)
"""Contrastive-center loss on 8 Trainium2 NeuronCores.

Math: with D[b,c] = ||feat_b - w_c||^2,
  intra = sum_b D[b, label_b]
  total = sum_{b,c} D[b,c] = C*sum_b||f_b||^2 + B*sum_c||w_c||^2
                             - 2*(sum_b f_b)·(sum_c w_c)
  inter = total - intra
  loss  = (1/2/B) * intra / (inter + eps) / 0.1

Data-parallel over batch: each core reduces its 256-row shard of
feat/label (plus the replicated centers) to 8 partial scalars. The
(B,C) distance matrix is never materialized, and nothing on the
feat-dependent critical path is a matmul: while feat streams in, the
kernel precomputes from label+weight alone
  G_t[b,:]  = w[label_b]     (transposed label-mask matmul, bf16)
  colwB[:,:] = colsum(W) broadcast to 128 partitions (all-ones matmul)
  cnt, c2, colw
so each arriving feat tile only needs two elementwise multiplies (DVE)
  feat ∘ G_t      -> sum = sum_b feat_b·w_{label_b}   (intra cross)
  feat ∘ colwB    -> sum = colsum(feat)·colsum(W)     (total cross)
whose free-axis sums ride the scalar-engine activation accumulator
(AF.Copy), plus an exact fp32 Square for ||feat||^2. A final all-ones
matmul reduces the partition axis; the per-core [1,8] results are
combined on the host (the unshard step): linear sums and one scalar
division. bf16 only touches the small cross terms (~1e-6 relative on
the loss); the dominant square terms stay fp32.
"""

import numpy as np

import concourse.bacc as bacc
import concourse.tile as tile
from concourse import mybir
from concourse.bass_utils import run_bass_kernel_spmd

B, C, D = 2048, 100, 512
N_CORES = 8
BS = B // N_CORES  # 256 batch rows per core
P = 128
NT = BS // P  # 2 partition tiles per core
LAMBDA_C = 1.0
EPSILON = 1e-6
SCALE = LAMBDA_C / 2.0 / B / 0.1

f32 = mybir.dt.float32
bf16 = mybir.dt.bfloat16
i32 = mybir.dt.int32
AF = mybir.ActivationFunctionType
ALU = mybir.AluOpType

# combined / res columns
F2_0, CNTC2, SW_0, C2, PR_0, SW_1, PR_1, F2_1 = range(8)
NCOL = 8


def _emit(nc, tc, feat, weight, label, res_out):
    with (
        tc.tile_pool(name="singles", bufs=1) as singles,
        tc.tile_pool(name="work", bufs=2) as work,
        tc.tile_pool(name="pp", bufs=1, space="PSUM") as pp,
    ):
        # --- scalar-engine activation-table prefetch (overlaps the DMAs) ---
        dummy = singles.tile([1, 2], f32)
        nc.vector.memset(dummy[:, 0:1], 0.0)
        nc.scalar.activation(dummy[:, 1:2], dummy[:, 0:1], AF.Square)

        # --- input DMAs: w early (everything precomputed hangs off it) ---
        w_sb = singles.tile([C, D], f32)
        feat_sb = singles.tile([P, NT * D], f32)
        lab_i = singles.tile([P, NT], i32)
        nc.scalar.dma_start(out=lab_i[:], in_=label.rearrange("(t p) o -> p (t o)", p=P))
        nc.scalar.dma_start(out=feat_sb[:, D:], in_=feat[P:, :])
        nc.sync.dma_start(out=w_sb[:], in_=weight[:, :])
        nc.sync.dma_start(out=feat_sb[:, :D], in_=feat[:P, :])

        # --- constants ---
        ones_f = singles.tile([P, 1], f32)
        nc.vector.memset(ones_f[:], 1.0)
        ones_mat = singles.tile([P, P], bf16)
        nc.vector.memset(ones_mat[:], 1.0)
        iota_i = singles.tile([P, P], i32)
        nc.gpsimd.iota(iota_i[:], pattern=[[1, P]], base=0, channel_multiplier=0)
        iota_f = singles.tile([P, P], f32)
        nc.vector.tensor_copy(iota_f[:], iota_i[:])
        iotac_i = singles.tile([P, 1], i32)
        nc.gpsimd.iota(iotac_i[:], pattern=[[1, 1]], base=0, channel_multiplier=1)
        iotac_f = singles.tile([P, 1], f32)
        nc.vector.tensor_copy(iotac_f[:], iotac_i[:])
        ident = singles.tile([P, P], f32)
        nc.vector.tensor_scalar(
            ident[:], iota_f[:], iotac_f[:], None, op0=ALU.is_equal
        )
        combined = singles.tile([P, NCOL], f32)
        nc.vector.memset(combined[:], 0.0)
        lab_f = singles.tile([P, NT], f32)
        nc.vector.tensor_copy(lab_f[:], lab_i[:])

        # --- label-side precompute: masks, histogram, transposed masks ---
        cnt_ps = pp.tile([C, 1], f32)
        maskT_sb = singles.tile([C, NT * P], bf16)
        for t in range(NT):
            mask = work.tile([P, C], f32, name="mask")
            nc.vector.tensor_scalar(
                mask[:], iota_f[:, :C], lab_f[:, t : t + 1], None, op0=ALU.is_equal
            )
            nc.tensor.matmul(
                cnt_ps[:], mask[:], ones_f[:], start=(t == 0), stop=(t == NT - 1)
            )
            tp_ps = pp.tile([C, P], f32, name=f"tp{t}")
            nc.tensor.transpose(tp_ps[:], mask[:], ident[:])
            nc.vector.tensor_copy(maskT_sb[:, t * P : (t + 1) * P], tp_ps[:])

        # --- weight-side precompute (needs only w + labels) ---
        w_sq = singles.tile([C, D], f32)
        nc.scalar.activation(w_sq[:], w_sb[:], AF.Square, accum_out=combined[:C, C2 : C2 + 1])
        w_bf = singles.tile([C, D], bf16)
        nc.vector.tensor_copy(w_bf[:], w_sb[:])
        # colw broadcast across all 128 partitions: every row = colsum(W)
        colwB_ps = pp.tile([P, D], f32)
        nc.tensor.matmul(colwB_ps[:], ones_mat[:C, :], w_bf[:], start=True, stop=True)
        colwB_sb = singles.tile([P, D], f32)
        nc.vector.tensor_copy(colwB_sb[:], colwB_ps[:])
        # G_t[b,:] = w[label_b] for each batch tile
        G_ps = [pp.tile([P, D], f32, name=f"G{t}") for t in range(NT)]
        for t in range(NT):
            nc.tensor.matmul(
                G_ps[t][:], maskT_sb[:, t * P : (t + 1) * P], w_bf[:],
                start=True, stop=True,
            )
        nc.vector.tensor_mul(
            combined[:C, CNTC2 : CNTC2 + 1], cnt_ps[:], combined[:C, C2 : C2 + 1]
        )

        # --- feat-dependent path: per tile, two DVE mults; reduces split
        # scalar (f2 Square-accum, f·G Copy-accum) / DVE (f·colwB) ---
        sq = singles.tile([P, NT * D], f32)
        scr = [singles.tile([P, NT * D], f32, name=f"scr{t}") for t in range(NT)]
        dump = [singles.tile([P, NT * D], f32, name=f"dump{t}") for t in range(NT)]
        f2col = (F2_0, F2_1)
        swcol = (SW_0, SW_1)
        prcol = (PR_0, PR_1)
        for t in range(NT):
            ft = feat_sb[:, t * D : (t + 1) * D]
            nc.scalar.activation(
                sq[:, t * D : (t + 1) * D], ft, AF.Square,
                accum_out=combined[:, f2col[t] : f2col[t] + 1],
            )
            nc.vector.tensor_mul(scr[t][:, :D], ft, G_ps[t][:])
            nc.scalar.activation(
                dump[t][:, :D], scr[t][:, :D], AF.Copy,
                accum_out=combined[:, swcol[t] : swcol[t] + 1],
            )
            nc.gpsimd.tensor_mul(scr[t][:, D:], ft, colwB_sb[:])
            nc.vector.tensor_reduce(
                combined[:, prcol[t] : prcol[t] + 1], scr[t][:, D:],
                axis=mybir.AxisListType.X, op=ALU.add,
            )

        # --- partition-axis reduction, then ship the 8 scalars ---
        res_ps = pp.tile([1, NCOL], f32)
        nc.tensor.matmul(res_ps[:], ones_f[:], combined[:], start=True, stop=True)
        res_sb = singles.tile([1, NCOL], f32)
        nc.vector.tensor_copy(res_sb[:], res_ps[:])
        nc.sync.dma_start(out=res_out[:, :], in_=res_sb[:])


def build_bass(reps=1):
    nc = bacc.Bacc(None, target_bir_lowering=False, num_devices=N_CORES)
    feat = nc.dram_tensor("feat", [BS, D], f32, kind="ExternalInput")
    weight = nc.dram_tensor("weight", [C, D], f32, kind="ExternalInput")
    label = nc.dram_tensor("label", [BS, 1], i32, kind="ExternalInput")
    res = nc.dram_tensor("res", [1, NCOL], f32, kind="ExternalOutput")
    with tile.TileContext(nc) as tc:
        for _ in range(reps):
            _emit(nc, tc, feat[:, :], weight[:, :], label[:, :], res[:, :])
    nc.compile()
    return nc


_NC = None


def _get_nc():
    global _NC
    if _NC is None:
        _NC = build_bass()
    return _NC


def make_in_maps(feat, weight, label):
    feat = np.ascontiguousarray(np.asarray(feat), dtype=np.float32)
    weight = np.ascontiguousarray(np.asarray(weight), dtype=np.float32)
    lab = np.ascontiguousarray(np.asarray(label).astype(np.int32).reshape(B, 1))
    return [
        {
            "feat": feat[c * BS : (c + 1) * BS],
            "weight": weight,
            "label": lab[c * BS : (c + 1) * BS],
        }
        for c in range(N_CORES)
    ]


def kernel(feat, weight, label):
    nc = _get_nc()
    in_maps = make_in_maps(feat, weight, label)
    res = run_bass_kernel_spmd(nc, in_maps, list(range(N_CORES)))
    # Unshard: sum the per-core partial reductions, then form the loss.
    r = np.zeros(NCOL, dtype=np.float64)
    for c in range(N_CORES):
        r += np.asarray(res.results[c]["res"], dtype=np.float64).reshape(NCOL)
    f2 = r[F2_0] + r[F2_1]
    sw = r[SW_0] + r[SW_1]
    dot = r[PR_0] + r[PR_1]
    intra = f2 + r[CNTC2] - 2.0 * sw
    total = C * f2 + BS * r[C2] - 2.0 * dot
    loss = SCALE * intra / (total - intra + EPSILON)
    return np.float32(loss)
